# revision 8
# baseline (speedup 1.0000x reference)
"""GAT message-passing kernel for Trainium2, 8 NeuronCores, dst-partitioned.

Lane-aligned redesign (v2). Sized for N=50000, D=128, H=4, C=16, ED=64 but
parameterized so a tiny config can run in CoreSim.

Strategy:
 - Fold attention vectors into the linear weights on host (tiny matmuls):
   a_src = x @ u_src.T, a_dst = x @ u_dst.T, a_edge = edge_attr @ v.T.
   Softmax over incoming edges is computed WITHOUT max-subtraction (logits
   are bounded, softmax is shift-invariant), so per-dst sums suffice.
 - Host sorts nodes by in-degree (desc) and deals rank r to stratum
   s = r // 1024, core c, lane p. Window s on core c holds 128 same-degree
   nodes; K_s = max degree in stratum s (identical across cores -> SPMD).
 - LANE ALIGNMENT: the j-th incoming edge of the node at lane p sits at
   partition p of edge-block j. Scatter-add over a window collapses to a
   free-axis reduction per partition -- no one-hot matmuls, no dma_gather,
   no index tables on device.
 - Host pre-gathers x[src] per edge slot (pure data movement) and ships it
   fp16-transposed; the device does all FLOPs: per-block matmuls compute
   [xh | a_src] per edge and a_edge via the paired-vT8 trick, ACT engine
   does exp/copies, DVE does softmax + weighted aggregation, TensorE also
   computes the per-window self rows [xh | a_src | a_dst].
 - Pad slots carry x_pad with a_src = -1e4 so exp underflows to exactly 0.
 - Self-loops (PyG GATConv: loop edge_attr = per-dst mean of incoming
   edge_attr) fold in at window close via the per-window a_edge sums.
"""

import math

import numpy as np

NCORES = 8
D_IN = 128
H_HEADS = 4
C_OUT = 16
HC = H_HEADS * C_OUT  # 64
ED_DIM = 64
NEG_SLOPE = 0.2
DUMMY_ASRC = -1.0e4  # pad slots: lrelu -> -2e3, exp -> 0 in f32
P = 128

TRACE = False       # set by test harness to capture an NTFF profile
LAST_RESULT = None  # BassKernelResults of the last traced run


class _Cfg:
    def __init__(self, nwl, ks):
        self.NWL = nwl                      # windows (= strata) per core
        self.KS = tuple(int(k) for k in ks)  # edge blocks per window
        self.NPS = tuple((k + 1) // 2 for k in self.KS)
        self.CUMK = np.concatenate([[0], np.cumsum(self.KS)]).astype(np.int64)
        self.CUMNP = np.concatenate([[0], np.cumsum(self.NPS)]).astype(np.int64)
        self.ECB = int(self.CUMK[-1])       # total edge blocks per core
        self.NPTOT = int(self.CUMNP[-1])
        self.KMAX = int(max(self.KS))

    def key(self):
        return (self.NWL, self.KS)


def _fold_weights(W, W_edge, att_src, att_dst, att_edge):
    H, C = att_src.shape
    D = W.shape[1]
    ED = W_edge.shape[1]
    u_src = np.einsum("hc,hcd->hd", att_src, W.reshape(H, C, D))
    u_dst = np.einsum("hc,hcd->hd", att_dst, W.reshape(H, C, D))
    v = np.einsum("hc,hcd->hd", att_edge, W_edge.reshape(H, C, ED))
    Wall = np.zeros((D, HC + H), np.float16)     # [W.T | u_src.T]
    Wall[:, :HC] = W.T
    Wall[:, HC:] = u_src.T
    Wself = np.zeros((D, HC + 2 * H), np.float16)  # [W.T | u_src.T | u_dst.T]
    Wself[:, :HC] = W.T
    Wself[:, HC:HC + H] = u_src.T
    Wself[:, HC + H:] = u_dst.T
    # vT8: rows 0:ED -> [v.T | 0], rows ED:2ED -> [0 | v.T] (paired matmul)
    vT8 = np.zeros((2 * ED, 2 * H), np.float16)
    vT8[:ED, :H] = v.T
    vT8[ED:, H:] = v.T
    # pad-slot x: a_src = u_src . x_pad == DUMMY_ASRC (any head), finite xh
    usum = u_src.sum(axis=0)
    x_pad = (usum * (DUMMY_ASRC / np.dot(usum, usum))).astype(np.float16)
    return Wall, Wself, vT8, x_pad, u_src


def _prep(x, src, dst, edge_attr, x_pad):
    """Degree-sorted lane packing; per-core fp16 input maps."""
    n = x.shape[0]
    nwl = math.ceil(n / (P * NCORES))
    spp = P * NCORES                  # nodes per stratum
    nslots = nwl * spp

    deg = np.bincount(dst, minlength=n).astype(np.int64)
    degp = np.zeros(nslots, np.int64)
    degp[:n] = deg
    order = np.argsort(-degp, kind="stable")      # rank -> node (virtual >= n)
    degs_sorted = degp[order]
    ks = np.maximum(1, degs_sorted[np.arange(nwl) * spp])
    cfg = _Cfg(nwl, ks)

    rank_of = np.empty(nslots, np.int64)
    rank_of[order] = np.arange(nslots)
    s_all = rank_of // spp
    q_all = rank_of % spp
    c_all = q_all // P
    p_all = q_all % P
    # out row of node i in core c_all[i]'s output
    winpos = (c_all * nwl + s_all) * P + p_all     # global out_ws row

    # --- edge placement ---
    er = rank_of[dst]                              # dst rank per edge
    eorder = np.argsort(er, kind="stable")
    er_s = er[eorder]
    offs = np.concatenate([[0], np.cumsum(degs_sorted)])
    j_e = np.arange(len(er_s), dtype=np.int64) - offs[er_s]
    s_e = er_s // spp
    c_e = (er_s % spp) // P
    p_e = er_s % P
    blk = cfg.CUMK[s_e] + j_e                      # block index within core
    slot = blk * P + p_e
    pcol = (cfg.CUMNP[s_e] + j_e // 2) * P + p_e   # eaT2 pair column
    half = (j_e % 2).astype(bool)
    src_e = src[eorder]

    x16 = x.astype(np.float16)
    ea16 = edge_attr.astype(np.float16)
    x16e = x16[src_e]

    in_maps = []
    for c in range(NCORES):
        m = c_e == c
        perm = np.full(cfg.ECB * P, -1, np.int64)
        perm[slot[m]] = np.arange(len(src_e))[m]
        xsrcT = np.empty((cfg.ECB * P, D_IN), np.float16)
        xsrcT[:] = x_pad[None, :]
        sel = perm >= 0
        xsrcT[sel] = x16e[perm[sel]]
        xsrcT = np.ascontiguousarray(xsrcT.T)      # [128, ECB*P]

        eaT2 = np.zeros((2 * ED_DIM, cfg.NPTOT * P), np.float16)
        m0 = m & ~half
        m1 = m & half
        eaT2[:ED_DIM, pcol[m0]] = ea16[eorder[m0]].T
        eaT2[ED_DIM:, pcol[m1]] = ea16[eorder[m1]].T

        xselfT = np.zeros((D_IN, nwl * P), np.float16)
        nodes_c = np.where((c_all == c) & (np.arange(nslots) < n))[0]
        xselfT[:, s_all[nodes_c] * P + p_all[nodes_c]] = x16[nodes_c].T

        invcnt = np.ones((P, nwl), np.float32)
        invcnt[p_all[nodes_c], s_all[nodes_c]] = (
            1.0 / np.maximum(deg[nodes_c], 1)).astype(np.float32)

        in_maps.append(dict(xsrcT=xsrcT, eaT2=eaT2, xselfT=xselfT,
                            invcnt=invcnt))
    meta = dict(winpos=winpos[:n], cfg=cfg)
    return cfg, in_maps, meta


def _build_nc(cfg):
    import concourse.bass as bass  # noqa: F401
    import concourse.tile as tile
    from concourse import bacc, mybir
    from contextlib import ExitStack

    f32 = mybir.dt.float32
    f16 = mybir.dt.float16
    AF = mybir.ActivationFunctionType
    OP = mybir.AluOpType
    NWL, KS, NPS = cfg.NWL, cfg.KS, cfg.NPS
    CUMK, CUMNP = cfg.CUMK, cfg.CUMNP
    KMAX = cfg.KMAX
    NPMAX = max(NPS)
    UH = H_HEADS
    TW = HC + UH          # 68: [xh | a_src]
    TS = HC + 2 * UH      # 72: [xh | a_src | a_dst]
    GB = 7                # xh blocks per PSUM bank (7*68*4B = 1904 <= 2048)

    nc = bacc.Bacc("TRN2", target_bir_lowering=False, debug=False,
                   num_devices=NCORES)
    xsrcT = nc.dram_tensor("xsrcT", [D_IN, cfg.ECB * P], f16,
                           kind="ExternalInput").ap()
    eaT2 = nc.dram_tensor("eaT2", [2 * ED_DIM, cfg.NPTOT * P], f16,
                          kind="ExternalInput").ap()
    xselfT = nc.dram_tensor("xselfT", [D_IN, NWL * P], f16,
                            kind="ExternalInput").ap()
    Wall = nc.dram_tensor("Wall", [D_IN, TW], f16, kind="ExternalInput").ap()
    Wself = nc.dram_tensor("Wself", [D_IN, TS], f16, kind="ExternalInput").ap()
    vT8 = nc.dram_tensor("vT8", [2 * ED_DIM, 2 * UH], f16,
                         kind="ExternalInput").ap()
    invcnt = nc.dram_tensor("invcnt", [P, NWL], f32, kind="ExternalInput").ap()
    out = nc.dram_tensor("out", [NWL * P, HC], f32, kind="ExternalOutput").ap()

    with tile.TileContext(nc) as tc, ExitStack() as ctx:
        cpool = ctx.enter_context(tc.tile_pool(name="const", bufs=1))
        spool = ctx.enter_context(tc.tile_pool(name="xself", bufs=3))
        xpool = ctx.enter_context(tc.tile_pool(name="xsrc", bufs=3))
        epool = ctx.enter_context(tc.tile_pool(name="ea", bufs=3))
        gpool = ctx.enter_context(tc.tile_pool(name="G", bufs=3))
        mpool = ctx.enter_context(tc.tile_pool(name="msg", bufs=2))
        wpool = ctx.enter_context(tc.tile_pool(name="work", bufs=3))
        opool = ctx.enter_context(tc.tile_pool(name="outw", bufs=3))
        psG = ctx.enter_context(tc.tile_pool(name="ps_g", bufs=4, space="PSUM"))
        psAE = ctx.enter_context(tc.tile_pool(name="ps_ae", bufs=2, space="PSUM"))
        psS = ctx.enter_context(tc.tile_pool(name="ps_s", bufs=2, space="PSUM"))

        Wall_sb = cpool.tile([P, TW], f16)
        nc.sync.dma_start(Wall_sb[:], Wall[:])
        Wself_sb = cpool.tile([P, TS], f16)
        nc.sync.dma_start(Wself_sb[:], Wself[:])
        vT8_sb = cpool.tile([2 * ED_DIM, 2 * UH], f16)
        nc.sync.dma_start(vT8_sb[:], vT8[:])
        invcnt_sb = cpool.tile([P, NWL], f32)
        nc.sync.dma_start(invcnt_sb[:], invcnt[:])
        selfT_sb = cpool.tile([P, NWL * TS], f32)
        asum_sb = cpool.tile([P, NWL * UH], f32)

        # ---- phase S: per-window self rows [xh | a_src | a_dst] ----
        for s in range(NWL):
            xs = spool.tile([P, P], f16, tag="xself")
            nc.gpsimd.dma_start(xs[:], xselfT[:, s * P:(s + 1) * P])
            ps = psS.tile([P, TS], f32)
            nc.tensor.matmul(out=ps[:], lhsT=xs[:], rhs=Wself_sb[:],
                             start=True, stop=True)
            nc.scalar.activation(selfT_sb[:, s * TS:(s + 1) * TS], ps[:],
                                 AF.Copy)
        # asum = a_src_self + a_dst_self, all windows at once
        selfT3 = selfT_sb[:].rearrange("p (s u) -> p s u", u=TS)
        nc.vector.tensor_tensor(
            out=asum_sb[:].rearrange("p (s u) -> p s u", u=UH),
            in0=selfT3[:, :, HC:HC + UH], in1=selfT3[:, :, HC + UH:HC + 2 * UH],
            op=OP.add)

        # ---- phase B: per-window edge math ----
        for s in range(NWL):
            K = KS[s]
            NP_ = NPS[s]
            xs = xpool.tile([P, KMAX * P], f16, tag="xs")
            nc.sync.dma_start(xs[:, :K * P],
                              xsrcT[:, CUMK[s] * P:(CUMK[s] + K) * P])
            eat = epool.tile([2 * ED_DIM, NPMAX * P], f16, tag="eat")
            nc.sync.dma_start(eat[:, :NP_ * P],
                              eaT2[:, CUMNP[s] * P:(CUMNP[s] + NP_) * P])

            # a_edge via paired matmuls -> [P, K*4] block-major in PSUM
            ae_ps = psAE.tile([P, NPMAX * 2 * UH], f32)
            for t in range(NP_):
                nc.tensor.matmul(out=ae_ps[:, t * 8:(t + 1) * 8],
                                 lhsT=eat[:, t * P:(t + 1) * P],
                                 rhs=vT8_sb[:], start=True, stop=True)
            # AE copy to SBUF, h-major
            ae_sb = wpool.tile([P, UH * KMAX], f16, tag="ae")
            ae3 = ae_sb[:].rearrange("p (u k) -> p u k", k=KMAX)
            aeps3 = ae_ps[:, :K * UH].rearrange("p (k u) -> p u k", u=UH)
            nc.scalar.activation(ae3[:, :, :K], aeps3[:], AF.Copy)

            # [xh | a_src] per edge, GB blocks per PSUM bank
            G = gpool.tile([P, KMAX * TW], f16, tag="G")
            for c0 in range(0, K, GB):
                nb = min(GB, K - c0)
                g_ps = psG.tile([P, GB * TW], f32)
                for j in range(nb):
                    nc.tensor.matmul(
                        out=g_ps[:, j * TW:(j + 1) * TW],
                        lhsT=xs[:, (c0 + j) * P:(c0 + j + 1) * P],
                        rhs=Wall_sb[:], start=True, stop=True)
                nc.scalar.activation(G[:, c0 * TW:(c0 + nb) * TW],
                                     g_ps[:, :nb * TW], AF.Copy)
            G3 = G[:, :K * TW].rearrange("p (k u) -> p k u", u=TW)
            GT = G[:, :K * TW].rearrange("p (k u) -> p u k", u=TW)

            # alpha = a_src + a_edge + a_dst(lane), h-major; lrelu = max(x,.2x)
            al = wpool.tile([P, UH * KMAX], f32, tag="al")
            al3 = al[:].rearrange("p (u k) -> p u k", k=KMAX)
            nc.vector.tensor_tensor(
                out=al3[:, :, :K], in0=GT[:, HC:HC + UH, :],
                in1=ae3[:, :, :K], op=OP.add)
            adst = selfT_sb[:, s * TS + HC + UH:s * TS + HC + 2 * UH]
            nc.vector.tensor_tensor(
                out=al3[:, :, :K], in0=al3[:, :, :K],
                in1=adst.unsqueeze(2).broadcast_to([P, UH, K]), op=OP.add)
            nc.vector.scalar_tensor_tensor(
                out=al3[:, :, :K], in0=al3[:, :, :K], scalar=NEG_SLOPE,
                in1=al3[:, :, :K], op0=OP.mult, op1=OP.max)
            ex = wpool.tile([P, UH * KMAX], f16, tag="ex")
            ex3 = ex[:].rearrange("p (u k) -> p u k", k=KMAX)
            nc.scalar.activation(ex3[:, :, :K], al3[:, :, :K], AF.Exp)

            den = wpool.tile([P, UH], f32, tag="den")
            nc.vector.tensor_reduce(
                out=den[:], in_=ex3[:, :, :K],
                axis=mybir.AxisListType.X, op=OP.add)
            lae = wpool.tile([P, UH], f32, tag="lae")
            nc.vector.tensor_reduce(
                out=lae[:], in_=aeps3[:],
                axis=mybir.AxisListType.X, op=OP.add)
            nc.vector.tensor_scalar(out=lae[:], in0=lae[:],
                                    scalar1=invcnt_sb[:, s:s + 1],
                                    scalar2=None, op0=OP.mult)

            # weighted messages, then contiguous fp16 halving-tree over blocks
            msg = mpool.tile([P, KMAX * HC], f16, tag="msg")
            gx4 = G3[:, :, 0:HC].rearrange("p k (h c) -> p k h c", c=C_OUT)
            ex4 = ex3[:, :, :K].rearrange("p u k -> p k u").unsqueeze(
                3).broadcast_to([P, K, UH, C_OUT])
            m4 = msg[:, :K * HC].rearrange("p (k h c) -> p k h c",
                                           h=UH, c=C_OUT)
            nc.vector.tensor_tensor(out=m4, in0=gx4, in1=ex4, op=OP.mult)
            cur = K
            while cur > 1:
                top = cur - cur // 2
                half = cur // 2
                nc.vector.tensor_tensor(
                    out=msg[:, :half * HC], in0=msg[:, :half * HC],
                    in1=msg[:, top * HC:cur * HC], op=OP.add)
                cur = top
            oacc = msg[:, 0:HC]

            # ---- window close: self-loop term + normalization ----
            selfw = selfT_sb[:, s * TS:(s + 1) * TS]
            asf = wpool.tile([P, UH], f32, tag="asf")
            nc.vector.tensor_tensor(
                out=asf[:], in0=asum_sb[:, s * UH:(s + 1) * UH],
                in1=lae[:], op=OP.add)
            esf = wpool.tile([P, UH], f32, tag="esf")
            nc.vector.scalar_tensor_tensor(
                out=esf[:], in0=asf[:], scalar=NEG_SLOPE, in1=asf[:],
                op0=OP.mult, op1=OP.max)
            nc.scalar.activation(esf[:], esf[:], AF.Exp)
            # den += exp(alpha_self) + 1e-30 (keeps empty lanes finite)
            nc.vector.scalar_tensor_tensor(
                out=den[:], in0=esf[:], scalar=1e-30, in1=den[:],
                op0=OP.add, op1=OP.add)
            rec = wpool.tile([P, UH], f32, tag="rec")
            nc.vector.reciprocal(rec[:], den[:])
            ot = opool.tile([P, HC], f32, tag="ot")
            esb = esf[:].unsqueeze(2).broadcast_to([P, UH, C_OUT])
            nc.vector.tensor_tensor(
                out=ot[:].rearrange("p (h c) -> p h c", c=C_OUT),
                in0=selfw[:, 0:HC].rearrange("p (h c) -> p h c", c=C_OUT),
                in1=esb, op=OP.mult)
            nc.vector.tensor_tensor(out=ot[:], in0=ot[:], in1=oacc,
                                    op=OP.add)
            recb = rec[:].unsqueeze(2).broadcast_to([P, UH, C_OUT])
            nc.vector.tensor_tensor(
                out=ot[:].rearrange("p (h c) -> p h c", c=C_OUT),
                in0=ot[:].rearrange("p (h c) -> p h c", c=C_OUT),
                in1=recb, op=OP.mult)
            nc.gpsimd.dma_start(out[s * P:(s + 1) * P, :], ot[:])

    nc.compile()
    return nc


_NC_CACHE = {}


def _get_nc(cfg):
    k = cfg.key()
    if k not in _NC_CACHE:
        _NC_CACHE[k] = _build_nc(cfg)
    return _NC_CACHE[k]


def kernel(**inputs):
    x = np.asarray(inputs["x"], dtype=np.float32)
    ei = np.asarray(inputs["edge_index"])
    ea = np.asarray(inputs["edge_attr"], dtype=np.float32)
    W = np.asarray(inputs["W"], dtype=np.float32)
    W_edge = np.asarray(inputs["W_edge"], dtype=np.float32)
    att_src = np.asarray(inputs["att_src"], dtype=np.float32)
    att_dst = np.asarray(inputs["att_dst"], dtype=np.float32)
    att_edge = np.asarray(inputs["att_edge"], dtype=np.float32)
    bias = np.asarray(inputs["bias"], dtype=np.float32)

    src = ei[0].astype(np.int64)
    dst = ei[1].astype(np.int64)
    Wall, Wself, vT8, x_pad, _ = _fold_weights(
        W, W_edge, att_src, att_dst, att_edge)

    cfg, in_maps, meta = _prep(x, src, dst, ea, x_pad)
    for m in in_maps:
        m["Wall"] = Wall
        m["Wself"] = Wself
        m["vT8"] = vT8
    nc = _get_nc(cfg)

    from concourse.bass_utils import run_bass_kernel_spmd
    res = run_bass_kernel_spmd(nc, in_maps, core_ids=list(range(NCORES)),
                               trace=TRACE)
    if TRACE:
        global LAST_RESULT
        LAST_RESULT = res

    out_ws = np.concatenate([res.results[c]["out"] for c in range(NCORES)],
                            axis=0)  # [nslots, HC] in window space
    out = out_ws[meta["winpos"]]
    return (out + bias[None, :]).astype(np.float32)
